# revision 17
# baseline (speedup 1.0000x reference)
"""Trainium2 Bass kernel for masked BasicBlock (conv3x3+BN+ReLU, gated, x2, residual).

Data-parallel over batch: 8 images -> 8 NeuronCores. Per core, NCHW [64,256,256]
in 8 row-strips of 32 rows.

Key algebraic fact: the gmax (maxpool3x3 of gate) multiply on h is redundant --
every pixel that survives the final `gate` multiply has all nine conv2-input
neighbors inside gmax==1, so unmasked h gives the identical output. So:
    h  = relu(bn1(conv1(x)))            (no mask)
    out= relu(bn2(conv2(h))*gate + x)

Each conv is computed per 4-row "quad" (pairs A=rows 4m..4m+1 on PSUM
partitions 0:64, B=rows 4m+2..4m+3 on partitions 64:128) with 9 matmuls
(the minimum for K<=128 packing):
  - input tiles hold (lower = row t, upper = row t+1) duplicated layouts so a
    K=128 matmul covers two adjacent row-taps at once;
  - per dx: (a) K=128 -> A (taps ky=0,1), (c) K=128 -> B (taps ky=1,2), and
    (b) K=64, M=128 covering A's ky=2 and B's ky=0 simultaneously.
x arrives from the host already in the shifted-dup bf16 layout; the gate
arrives broadcast to the quad layout (bf16, values 0/1, DVE 2x-mode mult);
conv1's h is rearranged into the dup layout with 2x4 strided SBUF DMAs per
strip (each run spans two adjacent Hs slots incl. interior zero pads) plus 2
boundary-row DMAs between neighbor strips (no halo recompute).
Output is stored bf16 and upcast to f32 on the host.
"""
import sys

sys.path.insert(0, '/opt/trn_rl_repo')

import numpy as np
import ml_dtypes

BF16 = ml_dtypes.bfloat16

B, C, H, W = 8, 64, 256, 256
WP = W + 2            # padded row width (zero col at 0 and 257)
R = 32                # rows per strip
NS = H // R           # strips
NQ = R // 4           # quads per strip
XSL = 33              # x_dup slots per strip
HSL = 35              # h dup slots per strip (33 used + 2 scratch for strided views)

_CACHE = {}


def _build():
    import concourse.bacc as bacc_mod
    import concourse.tile as tile
    import concourse.mybir as mybir

    dt = mybir.dt
    nc = bacc_mod.Bacc()

    xd = nc.dram_tensor("xd", [128, H + 4, WP], dt.bfloat16, kind="ExternalInput")
    gq_d = nc.dram_tensor("gq", [128, H // 4, 512], dt.bfloat16, kind="ExternalInput")
    wa1_d = nc.dram_tensor("wa1", [128, 3, 64], dt.bfloat16, kind="ExternalInput")
    wb1_d = nc.dram_tensor("wb1", [64, 3, 128], dt.bfloat16, kind="ExternalInput")
    wc1_d = nc.dram_tensor("wc1", [128, 3, 64], dt.bfloat16, kind="ExternalInput")
    wa2_d = nc.dram_tensor("wa2", [128, 3, 64], dt.bfloat16, kind="ExternalInput")
    wb2_d = nc.dram_tensor("wb2", [64, 3, 128], dt.bfloat16, kind="ExternalInput")
    wc2_d = nc.dram_tensor("wc2", [128, 3, 64], dt.bfloat16, kind="ExternalInput")
    sb1_d = nc.dram_tensor("sb1", [128, 2], dt.float32, kind="ExternalInput")
    sb2_d = nc.dram_tensor("sb2", [128, 2], dt.float32, kind="ExternalInput")
    o_d = nc.dram_tensor("o", [C, H, W], dt.bfloat16, kind="ExternalOutput")

    RELU = mybir.ActivationFunctionType.Relu
    IDENT = mybir.ActivationFunctionType.Identity

    with tile.TileContext(nc) as tc:
        with (
            tc.tile_pool(name="const", bufs=1) as cpool,
            tc.tile_pool(name="xs", bufs=3) as xpool,
            tc.tile_pool(name="hs", bufs=3) as hpool,
            tc.tile_pool(name="hp", bufs=2) as hppool,
            tc.tile_pool(name="gq", bufs=2) as gqpool,
            tc.tile_pool(name="ov", bufs=2) as ovpool,
            tc.tile_pool(name="u2", bufs=3) as upool,
            tc.tile_pool(name="tv", bufs=3) as tvpool,
            tc.tile_pool(name="ps1", bufs=4, space="PSUM") as ps1,
            tc.tile_pool(name="ps2", bufs=4, space="PSUM") as ps2,
        ):
            wa1 = cpool.tile([128, 3, 64], dt.bfloat16)
            wb1 = cpool.tile([64, 3, 128], dt.bfloat16)
            wc1 = cpool.tile([128, 3, 64], dt.bfloat16)
            wa2 = cpool.tile([128, 3, 64], dt.bfloat16)
            wb2 = cpool.tile([64, 3, 128], dt.bfloat16)
            wc2 = cpool.tile([128, 3, 64], dt.bfloat16)
            sb1 = cpool.tile([128, 2], dt.float32)
            sb2 = cpool.tile([128, 2], dt.float32)
            # weights via SWDGE (Pool) so the first Xs load owns HWDGE at t=0
            for t, d in ((wa1, wa1_d), (wb1, wb1_d), (wc1, wc1_d),
                         (wa2, wa2_d), (wb2, wb2_d), (wc2, wc2_d),
                         (sb1, sb1_d), (sb2, sb2_d)):
                nc.gpsimd.dma_start(t[:], d[:])

            Xs_t = [None] * NS
            Hs_t = [None] * NS
            HP_t = [None] * NS

            for it in range(NS + 1):
                # ---------------- conv1 of strip s = it ----------------
                if it < NS:
                    s = it
                    r0 = s * R
                    Xs = xpool.tile([128, XSL, WP], dt.bfloat16, tag="Xs")
                    Xs_t[s] = Xs
                    if s == 0:
                        # chunked first load so quad-0 matmuls start early
                        for c0, c1 in ((0, 5), (5, 17), (17, XSL)):
                            nc.sync.dma_start(Xs[:, c0:c1, :],
                                              xd[:, r0 + 1 + c0:r0 + 1 + c1, :])
                    else:
                        nc.sync.dma_start(Xs[:], xd[:, r0 + 1:r0 + 1 + XSL, :])

                    Hs = hpool.tile([128, HSL, WP], dt.bfloat16, tag="Hs")
                    Hs_t[s] = Hs
                    nc.vector.memset(Hs[:, :, 0:1], 0)
                    nc.vector.memset(Hs[:, :, 257:258], 0)
                    if s == 0:
                        nc.vector.memset(Hs[0:64, 0:1, :], 0)       # h[-1] = 0
                    if s == NS - 1:
                        nc.vector.memset(Hs[64:128, 32:33, :], 0)   # h[256] = 0

                    # HP: per-quad staging with 2 zero gap cols between the
                    # row-pair halves so distribute DMA runs span 2 Hs slots
                    HP = hppool.tile([128, NQ, 516], dt.bfloat16, tag="HP")
                    HP_t[s] = HP
                    nc.vector.memset(HP[:, :, 256:258], 0)

                if it >= 1:
                    sp = it - 1
                    r0p = sp * R
                    GQ = gqpool.tile([128, NQ, 512], dt.bfloat16, tag="GQ")
                    nc.sync.dma_start(GQ[:], gq_d[:, sp * NQ:(sp + 1) * NQ, :])
                    OV = ovpool.tile([128, NQ, 512], dt.bfloat16, tag="OV")
                    Hsp = Hs_t[sp]
                    Xsp = Xs_t[sp]

                for mm in range(NQ):
                    if it < NS:
                        # conv1 quad mm: 9 matmuls into acc, then BN+relu -> HP
                        acc = ps1.tile([128, 512], dt.float32, tag="ps1")
                        l = 4 * mm
                        for dx in range(3):
                            nc.tensor.matmul(acc[0:64, :], wa1[:, dx, :],
                                             Xs[:, l:l + 2, dx:dx + 256],
                                             start=(dx == 0), stop=False,
                                             tile_position=(0, 0), skip_group_check=True)
                        for dx in range(3):
                            nc.tensor.matmul(acc[64:128, :], wc1[:, dx, :],
                                             Xs[:, l + 3:l + 5, dx:dx + 256],
                                             start=(dx == 0), stop=False,
                                             tile_position=(0, 64), skip_group_check=True)
                        for dx in range(3):
                            nc.tensor.matmul(acc[:, :], wb1[:, dx, :],
                                             Xs[0:64, l + 2:l + 4, dx:dx + 256],
                                             start=False, stop=(dx == 2),
                                             tile_position=(0, 0), skip_group_check=True)
                        hpq = HP[:, mm, :].rearrange("p (r w) -> p r w", r=2)
                        nc.scalar.activation(hpq[:, :, 0:256], acc[:], RELU,
                                             bias=sb1[:, 1:2], scale=sb1[:, 0:1])
                        if mm == 0 and it >= 1:
                            # h[r0] -> prev strip's Hs upper slot 32
                            nc.sync.dma_start(Hs_t[it - 1][64:128, 32:33, 1:257],
                                              HP[0:64, 0:1, 0:256])
                        if mm == 3 or mm == 7:
                            # distribute this half-strip of HP into the Hs dup
                            # layout: 4 DMAs whose runs span 2 adjacent Hs slots
                            # (data + interior zero pads, 514 cols each)
                            qlo, qhi = (0, 4) if mm == 3 else (4, 8)
                            nq2 = qhi - qlo
                            lo1 = Hs[0:64, 1 + 4 * qlo:1 + 4 * qhi, :].rearrange(
                                "c (q r) w -> c q (r w)", q=nq2)
                            lo2 = Hs[0:64, 3 + 4 * qlo:3 + 4 * qhi, :].rearrange(
                                "c (q r) w -> c q (r w)", q=nq2)
                            up1 = Hs[64:128, 4 * qlo:4 * qhi, :].rearrange(
                                "c (q r) w -> c q (r w)", q=nq2)
                            up2 = Hs[64:128, 2 + 4 * qlo:2 + 4 * qhi, :].rearrange(
                                "c (q r) w -> c q (r w)", q=nq2)
                            nc.sync.dma_start(lo1[:, :, 1:515], HP[0:64, qlo:qhi, 0:514])
                            nc.sync.dma_start(lo2[:, :, 1:515], HP[64:128, qlo:qhi, 0:514])
                            nc.sync.dma_start(up1[:, :, 1:515], HP[0:64, qlo:qhi, 0:514])
                            nc.sync.dma_start(up2[:, :, 1:515], HP[64:128, qlo:qhi, 0:514])

                    if it >= 1:
                        # conv2 quad mm of strip sp
                        acc2 = ps2.tile([128, 512], dt.float32, tag="ps2")
                        l = 4 * mm
                        for dx in range(3):
                            nc.tensor.matmul(acc2[0:64, :], wa2[:, dx, :],
                                             Hsp[:, l:l + 2, dx:dx + 256],
                                             start=(dx == 0), stop=False,
                                             tile_position=(0, 0), skip_group_check=True)
                        for dx in range(3):
                            nc.tensor.matmul(acc2[64:128, :], wc2[:, dx, :],
                                             Hsp[:, l + 3:l + 5, dx:dx + 256],
                                             start=(dx == 0), stop=False,
                                             tile_position=(0, 64), skip_group_check=True)
                        for dx in range(3):
                            nc.tensor.matmul(acc2[:, :], wb2[:, dx, :],
                                             Hsp[0:64, l + 2:l + 4, dx:dx + 256],
                                             start=False, stop=(dx == 2),
                                             tile_position=(0, 0), skip_group_check=True)
                        u2 = upool.tile([128, 512], dt.bfloat16, tag="u2")
                        nc.scalar.activation(u2[:], acc2[:], IDENT,
                                             bias=sb2[:, 1:2], scale=sb2[:, 0:1])
                        t = tvpool.tile([128, 512], dt.bfloat16, tag="t")
                        nc.vector.tensor_tensor(t[:], u2[:], GQ[:, mm, :],
                                                mybir.AluOpType.mult)
                        v = tvpool.tile([128, 512], dt.bfloat16, tag="v")
                        nc.vector.tensor_tensor(
                            v[0:64, :].rearrange("p (r w) -> p r w", r=2),
                            t[0:64, :].rearrange("p (r w) -> p r w", r=2),
                            Xsp[0:64, l + 1:l + 3, 1:257], mybir.AluOpType.add)
                        nc.vector.tensor_tensor(
                            v[64:128, :].rearrange("p (r w) -> p r w", r=2),
                            t[64:128, :].rearrange("p (r w) -> p r w", r=2),
                            Xsp[64:128, l + 2:l + 4, 1:257], mybir.AluOpType.add)
                        nc.vector.tensor_scalar_max(OV[:, mm, :], v[:], 0.0)

                if it < NS and it >= 1:
                    # boundary: h[r0-1] from prev HP into this strip's lower slot 0
                    nc.sync.dma_start(Hs[0:64, 0:1, 1:257],
                                      HP_t[it - 1][64:128, NQ - 1:NQ, 258:514])

                if it >= 1:
                    # store strip sp: A rows (4q,4q+1) then B rows (4q+2,4q+3)
                    od = o_d[:, r0p:r0p + R, :].rearrange("c (q r) w -> c q (r w)", q=NQ)
                    if it == NS:
                        # last strip: per-quad stores so the drain tail is short
                        for mm in range(NQ):
                            nc.gpsimd.dma_start(od[:, mm:mm + 1, 0:512],
                                                OV[0:64, mm:mm + 1, :])
                            nc.gpsimd.dma_start(od[:, mm:mm + 1, 512:1024],
                                                OV[64:128, mm:mm + 1, :])
                    else:
                        nc.gpsimd.dma_start(od[:, :, 0:512], OV[0:64, :, :])
                        nc.gpsimd.dma_start(od[:, :, 512:1024], OV[64:128, :, :])
    nc.finalize()
    return nc


def _host_prep(x, gate, w1, scale1, bias1, w2, scale2, bias2):
    # x_dup: [128, 260, 258] bf16 per image; slot g: lower = x[g-2], upper = x[g-1]
    xp = np.zeros((B, C, H + 5, WP), np.float32)
    xp[:, :, 2:2 + H, 1:257] = x
    x_dup = np.concatenate([xp[:, :, 0:H + 4, :], xp[:, :, 1:H + 5, :]], axis=1)
    x_dup = x_dup.astype(BF16)

    # gate quad layout: [128, 64, 512] fp8; 0/1 exact in fp8.
    # partitions 0:64 = gate rows (4m,4m+1); 64:128 = rows (4m+2,4m+3)
    g = gate[:, 0].reshape(B, H // 4, 4, W)
    ga = g[:, :, 0:2].reshape(B, H // 4, 512)
    gb = g[:, :, 2:4].reshape(B, H // 4, 512)
    gq_img = [np.ascontiguousarray(np.concatenate(
        [np.broadcast_to(ga[b][None], (64, H // 4, 512)),
         np.broadcast_to(gb[b][None], (64, H // 4, 512))], axis=0)).astype(BF16)
        for b in range(B)]

    def pack(w):
        wt = np.transpose(w, (1, 0, 2, 3))  # [ci, co, ky, kx]
        wa = np.empty((128, 3, 64), np.float32)
        wb = np.empty((64, 3, 128), np.float32)
        wc = np.empty((128, 3, 64), np.float32)
        for kx in range(3):
            wa[0:64, kx] = wt[:, :, 0, kx]
            wa[64:128, kx] = wt[:, :, 1, kx]
            wc[0:64, kx] = wt[:, :, 1, kx]
            wc[64:128, kx] = wt[:, :, 2, kx]
            wb[:, kx, 0:64] = wt[:, :, 2, kx]
            wb[:, kx, 64:128] = wt[:, :, 0, kx]
        return wa.astype(BF16), wb.astype(BF16), wc.astype(BF16)

    wa1, wb1, wc1 = pack(w1)
    wa2, wb2, wc2 = pack(w2)
    sb1 = np.stack([np.tile(scale1, 2), np.tile(bias1, 2)], axis=1).astype(np.float32)
    sb2 = np.stack([np.tile(scale2, 2), np.tile(bias2, 2)], axis=1).astype(np.float32)
    return x_dup, gq_img, wa1, wb1, wc1, wa2, wb2, wc2, sb1, sb2


def kernel(x, gate, w1, scale1, bias1, w2, scale2, bias2):
    from concourse.bass_utils import run_bass_kernel_spmd

    x = np.asarray(x, np.float32)
    gate = np.asarray(gate, np.float32)
    x_dup, gq_img, wa1, wb1, wc1, wa2, wb2, wc2, sb1, sb2 = _host_prep(
        x, gate, np.asarray(w1, np.float32), np.asarray(scale1, np.float32),
        np.asarray(bias1, np.float32), np.asarray(w2, np.float32),
        np.asarray(scale2, np.float32), np.asarray(bias2, np.float32))

    if 'nc' not in _CACHE:
        _CACHE['nc'] = _build()
    nc = _CACHE['nc']

    in_maps = []
    for b in range(B):
        in_maps.append({
            "xd": np.ascontiguousarray(x_dup[b]),
            "gq": gq_img[b],
            "wa1": wa1, "wb1": wb1, "wc1": wc1,
            "wa2": wa2, "wb2": wb2, "wc2": wc2,
            "sb1": sb1, "sb2": sb2,
        })
    res = run_bass_kernel_spmd(nc, in_maps, core_ids=list(range(B)))
    _CACHE['last_results'] = res
    out = np.stack([np.asarray(res.results[b]["o"]).astype(np.float32)
                    for b in range(B)], axis=0)
    return out


# revision 18
# speedup vs baseline: 1.0010x; 1.0010x over previous
"""Trainium2 Bass kernel for masked BasicBlock (conv3x3+BN+ReLU, gated, x2, residual).

Data-parallel over batch: 8 images -> 8 NeuronCores. Per core, NCHW [64,256,256]
in 8 row-strips of 32 rows.

Key algebraic fact: the gmax (maxpool3x3 of gate) multiply on h is redundant --
every pixel that survives the final `gate` multiply has all nine conv2-input
neighbors inside gmax==1, so unmasked h gives the identical output. So:
    h  = relu(bn1(conv1(x)))            (no mask)
    out= relu(bn2(conv2(h))*gate + x)

Each conv is computed per 4-row "quad" (pairs A=rows 4m..4m+1 on PSUM
partitions 0:64, B=rows 4m+2..4m+3 on partitions 64:128) with 9 matmuls
(the minimum for K<=128 packing):
  - input tiles hold (lower = row t, upper = row t+1) duplicated layouts so a
    K=128 matmul covers two adjacent row-taps at once;
  - per dx: (a) K=128 -> A (taps ky=0,1), (c) K=128 -> B (taps ky=1,2), and
    (b) K=64, M=128 covering A's ky=2 and B's ky=0 simultaneously.
x arrives from the host already in the shifted-dup bf16 layout; the gate
arrives broadcast to the quad layout (bf16, values 0/1, DVE 2x-mode mult);
conv1's h is rearranged into the dup layout with 2x4 strided SBUF DMAs per
strip (each run spans two adjacent Hs slots incl. interior zero pads) plus 2
boundary-row DMAs between neighbor strips (no halo recompute).
Output is stored bf16 and upcast to f32 on the host.
"""
import sys

sys.path.insert(0, '/opt/trn_rl_repo')

import numpy as np
import ml_dtypes

BF16 = ml_dtypes.bfloat16

B, C, H, W = 8, 64, 256, 256
WP = W + 2            # padded row width (zero col at 0 and 257)
R = 32                # rows per strip
NS = H // R           # strips
NQ = R // 4           # quads per strip
XSL = 33              # x_dup slots per strip
HSL = 35              # h dup slots per strip (33 used + 2 scratch for strided views)

_CACHE = {}


def _build():
    import concourse.bacc as bacc_mod
    import concourse.tile as tile
    import concourse.mybir as mybir

    dt = mybir.dt
    nc = bacc_mod.Bacc()

    xd = nc.dram_tensor("xd", [128, H + 4, WP], dt.bfloat16, kind="ExternalInput")
    gq_d = nc.dram_tensor("gq", [128, H // 4, 512], dt.bfloat16, kind="ExternalInput")
    wa1_d = nc.dram_tensor("wa1", [128, 3, 64], dt.bfloat16, kind="ExternalInput")
    wb1_d = nc.dram_tensor("wb1", [64, 3, 128], dt.bfloat16, kind="ExternalInput")
    wc1_d = nc.dram_tensor("wc1", [128, 3, 64], dt.bfloat16, kind="ExternalInput")
    wa2_d = nc.dram_tensor("wa2", [128, 3, 64], dt.bfloat16, kind="ExternalInput")
    wb2_d = nc.dram_tensor("wb2", [64, 3, 128], dt.bfloat16, kind="ExternalInput")
    wc2_d = nc.dram_tensor("wc2", [128, 3, 64], dt.bfloat16, kind="ExternalInput")
    sb1_d = nc.dram_tensor("sb1", [128, 2], dt.float32, kind="ExternalInput")
    sb2_d = nc.dram_tensor("sb2", [128, 2], dt.float32, kind="ExternalInput")
    o_d = nc.dram_tensor("o", [C, H, W], dt.bfloat16, kind="ExternalOutput")

    RELU = mybir.ActivationFunctionType.Relu
    IDENT = mybir.ActivationFunctionType.Identity

    with tile.TileContext(nc) as tc:
        with (
            tc.tile_pool(name="const", bufs=1) as cpool,
            tc.tile_pool(name="xs", bufs=3) as xpool,
            tc.tile_pool(name="hs", bufs=3) as hpool,
            tc.tile_pool(name="hp", bufs=2) as hppool,
            tc.tile_pool(name="gq", bufs=2) as gqpool,
            tc.tile_pool(name="ov", bufs=2) as ovpool,
            tc.tile_pool(name="u2", bufs=3) as upool,
            tc.tile_pool(name="tv", bufs=3) as tvpool,
            tc.tile_pool(name="ps1", bufs=4, space="PSUM") as ps1,
            tc.tile_pool(name="ps2", bufs=4, space="PSUM") as ps2,
        ):
            wa1 = cpool.tile([128, 3, 64], dt.bfloat16)
            wb1 = cpool.tile([64, 3, 128], dt.bfloat16)
            wc1 = cpool.tile([128, 3, 64], dt.bfloat16)
            wa2 = cpool.tile([128, 3, 64], dt.bfloat16)
            wb2 = cpool.tile([64, 3, 128], dt.bfloat16)
            wc2 = cpool.tile([128, 3, 64], dt.bfloat16)
            sb1 = cpool.tile([128, 2], dt.float32)
            sb2 = cpool.tile([128, 2], dt.float32)
            # weights via SWDGE (Pool) so the first Xs load owns HWDGE at t=0
            for t, d in ((wa1, wa1_d), (wb1, wb1_d), (wc1, wc1_d),
                         (wa2, wa2_d), (wb2, wb2_d), (wc2, wc2_d),
                         (sb1, sb1_d), (sb2, sb2_d)):
                nc.gpsimd.dma_start(t[:], d[:])

            Xs_t = [None] * NS
            Hs_t = [None] * NS
            HP_t = [None] * NS

            for it in range(NS + 1):
                # ---------------- conv1 of strip s = it ----------------
                if it < NS:
                    s = it
                    r0 = s * R
                    Xs = xpool.tile([128, XSL, WP], dt.bfloat16, tag="Xs")
                    Xs_t[s] = Xs
                    if s == 0:
                        # chunked first load so quad-0 matmuls start early
                        for c0, c1 in ((0, 5), (5, 17), (17, XSL)):
                            nc.sync.dma_start(Xs[:, c0:c1, :],
                                              xd[:, r0 + 1 + c0:r0 + 1 + c1, :])
                    else:
                        nc.sync.dma_start(Xs[:], xd[:, r0 + 1:r0 + 1 + XSL, :])

                    Hs = hpool.tile([128, HSL, WP], dt.bfloat16, tag="Hs")
                    Hs_t[s] = Hs
                    nc.vector.memset(Hs[:, :, 0:1], 0)
                    nc.vector.memset(Hs[:, :, 257:258], 0)
                    if s == 0:
                        nc.vector.memset(Hs[0:64, 0:1, :], 0)       # h[-1] = 0
                    if s == NS - 1:
                        nc.vector.memset(Hs[64:128, 32:33, :], 0)   # h[256] = 0

                    # HP: per-quad staging with 2 zero gap cols between the
                    # row-pair halves so distribute DMA runs span 2 Hs slots
                    HP = hppool.tile([128, NQ, 516], dt.bfloat16, tag="HP")
                    HP_t[s] = HP
                    nc.vector.memset(HP[:, :, 256:258], 0)

                if it >= 1:
                    sp = it - 1
                    r0p = sp * R
                    GQ = gqpool.tile([128, NQ, 512], dt.bfloat16, tag="GQ")
                    nc.sync.dma_start(GQ[:], gq_d[:, sp * NQ:(sp + 1) * NQ, :])
                    OV = ovpool.tile([128, NQ, 512], dt.bfloat16, tag="OV")
                    Hsp = Hs_t[sp]
                    Xsp = Xs_t[sp]

                for mm in range(NQ):
                    if it < NS:
                        # conv1 quad mm: 9 matmuls into acc, then BN+relu -> HP
                        acc = ps1.tile([128, 512], dt.float32, tag="ps1")
                        l = 4 * mm
                        for dx in range(3):
                            nc.tensor.matmul(acc[0:64, :], wa1[:, dx, :],
                                             Xs[:, l:l + 2, dx:dx + 256],
                                             start=(dx == 0), stop=False,
                                             tile_position=(0, 0), skip_group_check=True)
                        for dx in range(3):
                            nc.tensor.matmul(acc[64:128, :], wc1[:, dx, :],
                                             Xs[:, l + 3:l + 5, dx:dx + 256],
                                             start=(dx == 0), stop=False,
                                             tile_position=(0, 64), skip_group_check=True)
                        for dx in range(3):
                            nc.tensor.matmul(acc[:, :], wb1[:, dx, :],
                                             Xs[0:64, l + 2:l + 4, dx:dx + 256],
                                             start=False, stop=(dx == 2),
                                             tile_position=(0, 0), skip_group_check=True)
                        hpq = HP[:, mm, :].rearrange("p (r w) -> p r w", r=2)
                        nc.scalar.activation(hpq[:, :, 0:256], acc[:], RELU,
                                             bias=sb1[:, 1:2], scale=sb1[:, 0:1])
                        if mm == 0 and it >= 1:
                            # h[r0] -> prev strip's Hs upper slot 32
                            nc.sync.dma_start(Hs_t[it - 1][64:128, 32:33, 1:257],
                                              HP[0:64, 0:1, 0:256])
                        if mm == 3 or mm == 7:
                            # distribute this half-strip of HP into the Hs dup
                            # layout: 4 DMAs whose runs span 2 adjacent Hs slots
                            # (data + interior zero pads, 514 cols each)
                            qlo, qhi = (0, 4) if mm == 3 else (4, 8)
                            nq2 = qhi - qlo
                            lo1 = Hs[0:64, 1 + 4 * qlo:1 + 4 * qhi, :].rearrange(
                                "c (q r) w -> c q (r w)", q=nq2)
                            lo2 = Hs[0:64, 3 + 4 * qlo:3 + 4 * qhi, :].rearrange(
                                "c (q r) w -> c q (r w)", q=nq2)
                            up1 = Hs[64:128, 4 * qlo:4 * qhi, :].rearrange(
                                "c (q r) w -> c q (r w)", q=nq2)
                            up2 = Hs[64:128, 2 + 4 * qlo:2 + 4 * qhi, :].rearrange(
                                "c (q r) w -> c q (r w)", q=nq2)
                            nc.sync.dma_start(lo1[:, :, 1:515], HP[0:64, qlo:qhi, 0:514])
                            nc.sync.dma_start(lo2[:, :, 1:515], HP[64:128, qlo:qhi, 0:514])
                            nc.sync.dma_start(up1[:, :, 1:515], HP[0:64, qlo:qhi, 0:514])
                            nc.sync.dma_start(up2[:, :, 1:515], HP[64:128, qlo:qhi, 0:514])

                    if it >= 1:
                        # conv2 quad mm of strip sp
                        acc2 = ps2.tile([128, 512], dt.float32, tag="ps2")
                        l = 4 * mm
                        for dx in range(3):
                            nc.tensor.matmul(acc2[0:64, :], wa2[:, dx, :],
                                             Hsp[:, l:l + 2, dx:dx + 256],
                                             start=(dx == 0), stop=False,
                                             tile_position=(0, 0), skip_group_check=True)
                        for dx in range(3):
                            nc.tensor.matmul(acc2[64:128, :], wc2[:, dx, :],
                                             Hsp[:, l + 3:l + 5, dx:dx + 256],
                                             start=(dx == 0), stop=False,
                                             tile_position=(0, 64), skip_group_check=True)
                        for dx in range(3):
                            nc.tensor.matmul(acc2[:, :], wb2[:, dx, :],
                                             Hsp[0:64, l + 2:l + 4, dx:dx + 256],
                                             start=False, stop=(dx == 2),
                                             tile_position=(0, 0), skip_group_check=True)
                        u2 = upool.tile([128, 512], dt.bfloat16, tag="u2")
                        nc.scalar.activation(u2[:], acc2[:], IDENT,
                                             bias=sb2[:, 1:2], scale=sb2[:, 0:1])
                        t = tvpool.tile([128, 512], dt.bfloat16, tag="t")
                        nc.vector.tensor_tensor(t[:], u2[:], GQ[:, mm, :],
                                                mybir.AluOpType.mult)
                        v = tvpool.tile([128, 512], dt.bfloat16, tag="v")
                        nc.vector.tensor_tensor(
                            v[0:64, :].rearrange("p (r w) -> p r w", r=2),
                            t[0:64, :].rearrange("p (r w) -> p r w", r=2),
                            Xsp[0:64, l + 1:l + 3, 1:257], mybir.AluOpType.add)
                        nc.vector.tensor_tensor(
                            v[64:128, :].rearrange("p (r w) -> p r w", r=2),
                            t[64:128, :].rearrange("p (r w) -> p r w", r=2),
                            Xsp[64:128, l + 2:l + 4, 1:257], mybir.AluOpType.add)
                        nc.vector.tensor_scalar_max(OV[:, mm, :], v[:], 0.0)

                if it < NS and it >= 1:
                    # boundary: h[r0-1] from prev HP into this strip's lower slot 0
                    nc.sync.dma_start(Hs[0:64, 0:1, 1:257],
                                      HP_t[it - 1][64:128, NQ - 1:NQ, 258:514])

                if it >= 1:
                    # store strip sp: A rows (4q,4q+1) then B rows (4q+2,4q+3)
                    od = o_d[:, r0p:r0p + R, :].rearrange("c (q r) w -> c q (r w)", q=NQ)
                    if it == NS:
                        # last strip: per-quad stores (SP HWDGE; cheap issue)
                        # so the drain tail is short
                        for mm in range(NQ):
                            nc.sync.dma_start(od[:, mm:mm + 1, 0:512],
                                              OV[0:64, mm:mm + 1, :])
                            nc.sync.dma_start(od[:, mm:mm + 1, 512:1024],
                                              OV[64:128, mm:mm + 1, :])
                    else:
                        nc.gpsimd.dma_start(od[:, :, 0:512], OV[0:64, :, :])
                        nc.gpsimd.dma_start(od[:, :, 512:1024], OV[64:128, :, :])
    nc.finalize()
    return nc


def _host_prep(x, gate, w1, scale1, bias1, w2, scale2, bias2):
    # x_dup: [128, 260, 258] bf16 per image; slot g: lower = x[g-2], upper = x[g-1]
    xp = np.zeros((B, C, H + 5, WP), np.float32)
    xp[:, :, 2:2 + H, 1:257] = x
    x_dup = np.concatenate([xp[:, :, 0:H + 4, :], xp[:, :, 1:H + 5, :]], axis=1)
    x_dup = x_dup.astype(BF16)

    # gate quad layout: [128, 64, 512] fp8; 0/1 exact in fp8.
    # partitions 0:64 = gate rows (4m,4m+1); 64:128 = rows (4m+2,4m+3)
    g = gate[:, 0].reshape(B, H // 4, 4, W)
    ga = g[:, :, 0:2].reshape(B, H // 4, 512)
    gb = g[:, :, 2:4].reshape(B, H // 4, 512)
    gq_img = [np.ascontiguousarray(np.concatenate(
        [np.broadcast_to(ga[b][None], (64, H // 4, 512)),
         np.broadcast_to(gb[b][None], (64, H // 4, 512))], axis=0)).astype(BF16)
        for b in range(B)]

    def pack(w):
        wt = np.transpose(w, (1, 0, 2, 3))  # [ci, co, ky, kx]
        wa = np.empty((128, 3, 64), np.float32)
        wb = np.empty((64, 3, 128), np.float32)
        wc = np.empty((128, 3, 64), np.float32)
        for kx in range(3):
            wa[0:64, kx] = wt[:, :, 0, kx]
            wa[64:128, kx] = wt[:, :, 1, kx]
            wc[0:64, kx] = wt[:, :, 1, kx]
            wc[64:128, kx] = wt[:, :, 2, kx]
            wb[:, kx, 0:64] = wt[:, :, 2, kx]
            wb[:, kx, 64:128] = wt[:, :, 0, kx]
        return wa.astype(BF16), wb.astype(BF16), wc.astype(BF16)

    wa1, wb1, wc1 = pack(w1)
    wa2, wb2, wc2 = pack(w2)
    sb1 = np.stack([np.tile(scale1, 2), np.tile(bias1, 2)], axis=1).astype(np.float32)
    sb2 = np.stack([np.tile(scale2, 2), np.tile(bias2, 2)], axis=1).astype(np.float32)
    return x_dup, gq_img, wa1, wb1, wc1, wa2, wb2, wc2, sb1, sb2


def kernel(x, gate, w1, scale1, bias1, w2, scale2, bias2):
    from concourse.bass_utils import run_bass_kernel_spmd

    x = np.asarray(x, np.float32)
    gate = np.asarray(gate, np.float32)
    x_dup, gq_img, wa1, wb1, wc1, wa2, wb2, wc2, sb1, sb2 = _host_prep(
        x, gate, np.asarray(w1, np.float32), np.asarray(scale1, np.float32),
        np.asarray(bias1, np.float32), np.asarray(w2, np.float32),
        np.asarray(scale2, np.float32), np.asarray(bias2, np.float32))

    if 'nc' not in _CACHE:
        _CACHE['nc'] = _build()
    nc = _CACHE['nc']

    in_maps = []
    for b in range(B):
        in_maps.append({
            "xd": np.ascontiguousarray(x_dup[b]),
            "gq": gq_img[b],
            "wa1": wa1, "wb1": wb1, "wc1": wc1,
            "wa2": wa2, "wb2": wb2, "wc2": wc2,
            "sb1": sb1, "sb2": sb2,
        })
    res = run_bass_kernel_spmd(nc, in_maps, core_ids=list(range(B)))
    _CACHE['last_results'] = res
    out = np.stack([np.asarray(res.results[b]["o"]).astype(np.float32)
                    for b in range(B)], axis=0)
    return out


# revision 29
# speedup vs baseline: 1.1108x; 1.1097x over previous
"""Trainium2 Bass kernel for masked BasicBlock (conv3x3+BN+ReLU, gated, x2, residual).

Data-parallel over batch: 8 images -> 8 NeuronCores. Per core, NCHW [64,256,256]
in 8 row-strips of 32 rows.

Key algebraic fact: the gmax (maxpool3x3 of gate) multiply on h is redundant --
every pixel that survives the final `gate` multiply has all nine conv2-input
neighbors inside gmax==1, so unmasked h gives the identical output. So:
    h  = relu(bn1(conv1(x)))            (no mask)
    out= relu(bn2(conv2(h))*gate + x)

Each conv is computed per 4-row "quad" (pairs A=rows 4m..4m+1 on PSUM
partitions 0:64, B=rows 4m+2..4m+3 on partitions 64:128) with 9 matmuls
(the minimum for K<=128 packing):
  - input tiles hold (lower = row t, upper = row t+1) duplicated layouts so a
    K=128 matmul covers two adjacent row-taps at once;
  - per dx: (a) K=128 -> A (taps ky=0,1), (c) K=128 -> B (taps ky=1,2), and
    (b) K=64, M=128 covering A's ky=2 and B's ky=0 simultaneously.
x arrives from the host already in the shifted-dup bf16 layout; the gate
arrives broadcast to the quad layout (bf16, values 0/1, DVE 2x-mode mult);
conv1's h is rearranged into the dup layout with 2x4 strided SBUF DMAs per
strip (each run spans two adjacent Hs slots incl. interior zero pads) plus 2
boundary-row DMAs between neighbor strips (no halo recompute).
Output is stored bf16 and upcast to f32 on the host.
"""
import sys

sys.path.insert(0, '/opt/trn_rl_repo')

import numpy as np
import ml_dtypes

BF16 = ml_dtypes.bfloat16

B, C, H, W = 8, 64, 256, 256
WP = W + 2            # padded row width (zero col at 0 and 257)
R = 32                # rows per strip
NS = H // R           # strips
NQ = R // 4           # quads per strip
XSL = 33              # x_dup slots per strip
HSL = 35              # h dup slots per strip (33 used + 2 scratch for strided views)

_CACHE = {}


def _build():
    import concourse.bacc as bacc_mod
    import concourse.tile as tile
    import concourse.mybir as mybir

    dt = mybir.dt
    nc = bacc_mod.Bacc()

    xd = nc.dram_tensor("xd", [128, H + 4, WP], dt.bfloat16, kind="ExternalInput")
    xc_d = nc.dram_tensor("xc", [128, H // 4, 2, WP], dt.bfloat16, kind="ExternalInput")
    gq_d = nc.dram_tensor("gq", [128, H // 4, 512], dt.bfloat16, kind="ExternalInput")
    wa1_d = nc.dram_tensor("wa1", [128, 3, 64], dt.bfloat16, kind="ExternalInput")
    wb1_d = nc.dram_tensor("wb1", [64, 128], dt.bfloat16, kind="ExternalInput")
    wc1_d = nc.dram_tensor("wc1", [128, 3, 64], dt.bfloat16, kind="ExternalInput")
    wbc1_d = nc.dram_tensor("wbc1", [128, 128], dt.bfloat16, kind="ExternalInput")
    wa2_d = nc.dram_tensor("wa2", [128, 3, 64], dt.bfloat16, kind="ExternalInput")
    wb2_d = nc.dram_tensor("wb2", [64, 128], dt.bfloat16, kind="ExternalInput")
    wc2_d = nc.dram_tensor("wc2", [128, 3, 64], dt.bfloat16, kind="ExternalInput")
    wbc2_d = nc.dram_tensor("wbc2", [128, 128], dt.bfloat16, kind="ExternalInput")
    sb1_d = nc.dram_tensor("sb1", [128, 2], dt.float32, kind="ExternalInput")
    sb2_d = nc.dram_tensor("sb2", [128, 2], dt.float32, kind="ExternalInput")
    o_d = nc.dram_tensor("o", [C, H, W], dt.bfloat16, kind="ExternalOutput")

    RELU = mybir.ActivationFunctionType.Relu
    IDENT = mybir.ActivationFunctionType.Identity

    with tile.TileContext(nc) as tc:
        with (
            tc.tile_pool(name="const", bufs=1) as cpool,
            tc.tile_pool(name="xs", bufs=3) as xpool,
            tc.tile_pool(name="xc", bufs=2) as xcpool,
            tc.tile_pool(name="hs", bufs=2) as hpool,
            tc.tile_pool(name="hc", bufs=2) as hcpool,
            tc.tile_pool(name="hp", bufs=2) as hppool,
            tc.tile_pool(name="gq", bufs=2) as gqpool,
            tc.tile_pool(name="ov", bufs=2) as ovpool,
            tc.tile_pool(name="u2", bufs=2) as upool,
            tc.tile_pool(name="tv", bufs=3) as tvpool,
            tc.tile_pool(name="ps1", bufs=4, space="PSUM") as ps1,
            tc.tile_pool(name="ps2", bufs=4, space="PSUM") as ps2,
        ):
            wa1 = cpool.tile([128, 3, 64], dt.bfloat16)
            wb1 = cpool.tile([64, 128], dt.bfloat16)
            wc1 = cpool.tile([128, 3, 64], dt.bfloat16)
            wbc1 = cpool.tile([128, 128], dt.bfloat16)
            wa2 = cpool.tile([128, 3, 64], dt.bfloat16)
            wb2 = cpool.tile([64, 128], dt.bfloat16)
            wc2 = cpool.tile([128, 3, 64], dt.bfloat16)
            wbc2 = cpool.tile([128, 128], dt.bfloat16)
            sb1 = cpool.tile([128, 2], dt.float32)
            sb2 = cpool.tile([128, 2], dt.float32)
            # weights via SWDGE (Pool) so the first Xs load owns HWDGE at t=0
            for t, d in ((wa1, wa1_d), (wb1, wb1_d), (wc1, wc1_d), (wbc1, wbc1_d),
                         (wa2, wa2_d), (wb2, wb2_d), (wc2, wc2_d), (wbc2, wbc2_d),
                         (sb1, sb1_d), (sb2, sb2_d)):
                nc.gpsimd.dma_start(t[:], d[:])

            Xs_t = [None] * NS
            Xc_t = [None] * NS
            Hs_t = [None] * NS
            Hc_t = [None] * NS
            HP_t = [None] * NS

            for it in range(NS + 1):
                # ---------------- conv1 of strip s = it ----------------
                if it < NS:
                    s = it
                    r0 = s * R
                    Xs = xpool.tile([128, XSL, WP], dt.bfloat16, tag="Xs")
                    Xs_t[s] = Xs
                    if s == 0:
                        # chunked first load so quad-0 matmuls start early
                        for c0, c1 in ((0, 5), (5, 17), (17, XSL)):
                            nc.sync.dma_start(Xs[:, c0:c1, :],
                                              xd[:, r0 + 1 + c0:r0 + 1 + c1, :])
                    else:
                        nc.sync.dma_start(Xs[:], xd[:, r0 + 1:r0 + 1 + XSL, :])
                    # col-shift dup tile: rows (4m+1,4m+2) per quad, upper = +1 col
                    Xc = xcpool.tile([128, NQ, 2, WP], dt.bfloat16, tag="Xc")
                    Xc_t[s] = Xc
                    nc.scalar.dma_start(Xc[:], xc_d[:, s * NQ:(s + 1) * NQ, :, :])

                    Hs = hpool.tile([128, HSL, WP], dt.bfloat16, tag="Hs")
                    Hs_t[s] = Hs
                    nc.vector.memset(Hs[:, :, 0:1], 0)
                    nc.vector.memset(Hs[:, :, 257:258], 0)
                    Hc = hcpool.tile([128, NQ, 2, WP], dt.bfloat16, tag="Hc")
                    Hc_t[s] = Hc
                    nc.vector.memset(Hc[0:64, :, :, 0:1], 0)
                    nc.vector.memset(Hc[0:64, :, :, 257:258], 0)
                    nc.vector.memset(Hc[64:128, :, :, 256:258], 0)
                    if s == 0:
                        nc.vector.memset(Hs[0:64, 0:1, :], 0)       # h[-1] = 0
                    if s == NS - 1:
                        nc.vector.memset(Hs[64:128, 32:33, :], 0)   # h[256] = 0

                    # HP: per-quad staging with 2 zero gap cols between the
                    # row-pair halves so distribute DMA runs span 2 Hs slots
                    HP = hppool.tile([128, NQ, 516], dt.bfloat16, tag="HP")
                    HP_t[s] = HP
                    nc.vector.memset(HP[:, :, 256:258], 0)

                if it >= 1:
                    sp = it - 1
                    r0p = sp * R
                    GQ = gqpool.tile([128, NQ, 512], dt.bfloat16, tag="GQ")
                    nc.sync.dma_start(GQ[:], gq_d[:, sp * NQ:(sp + 1) * NQ, :])
                    OV = ovpool.tile([128, NQ, 512], dt.bfloat16, tag="OV")
                    Hsp = Hs_t[sp]
                    Xsp = Xs_t[sp]

                for mm in range(NQ):
                    if it < NS:
                        # conv1 quad mm: 9 matmuls into acc, then BN+relu -> HP
                        acc = ps1.tile([128, 512], dt.float32, tag="ps1")
                        l = 4 * mm
                        for dx in range(3):
                            nc.tensor.matmul(acc[0:64, :], wa1[:, dx, :],
                                             Xs[:, l:l + 2, dx:dx + 256],
                                             start=(dx == 0), stop=False,
                                             tile_position=(0, 0), skip_group_check=True)
                        for dx in range(3):
                            nc.tensor.matmul(acc[64:128, :], wc1[:, dx, :],
                                             Xs[:, l + 3:l + 5, dx:dx + 256],
                                             start=(dx == 0), stop=False,
                                             tile_position=(0, 64), skip_group_check=True)
                        # shared-window: col-dup 4-block matmul (kx0+kx1 for
                        # A-ky2 and B-ky0), then K=64 matmul for kx2
                        nc.tensor.matmul(acc[:, :], wbc1[:, :],
                                         Xc[:, mm, :, 0:256],
                                         start=False, stop=False,
                                         tile_position=(0, 0), skip_group_check=True)
                        nc.tensor.matmul(acc[:, :], wb1[:, :],
                                         Xs[0:64, l + 2:l + 4, 2:258],
                                         start=False, stop=True,
                                         tile_position=(0, 0), skip_group_check=True)
                        hpq = HP[:, mm, :].rearrange("p (r w) -> p r w", r=2)
                        nc.scalar.activation(hpq[:, :, 0:256], acc[:], RELU,
                                             bias=sb1[:, 1:2], scale=sb1[:, 0:1])
                        if mm == 0 and it >= 1:
                            # h[r0] -> prev strip's Hs upper slot 32
                            nc.sync.dma_start(Hs_t[it - 1][64:128, 32:33, 1:257],
                                              HP[0:64, 0:1, 0:256])
                        if mm == 3 or mm == 7:
                            # distribute this half-strip of HP into the Hs dup
                            # layout: 4 DMAs whose runs span 2 adjacent Hs slots
                            # (data + interior zero pads, 514 cols each)
                            qlo, qhi = (0, 4) if mm == 3 else (4, 8)
                            nq2 = qhi - qlo
                            lo1 = Hs[0:64, 1 + 4 * qlo:1 + 4 * qhi, :].rearrange(
                                "c (q r) w -> c q (r w)", q=nq2)
                            lo2 = Hs[0:64, 3 + 4 * qlo:3 + 4 * qhi, :].rearrange(
                                "c (q r) w -> c q (r w)", q=nq2)
                            up1 = Hs[64:128, 4 * qlo:4 * qhi, :].rearrange(
                                "c (q r) w -> c q (r w)", q=nq2)
                            up2 = Hs[64:128, 2 + 4 * qlo:2 + 4 * qhi, :].rearrange(
                                "c (q r) w -> c q (r w)", q=nq2)
                            nc.sync.dma_start(lo1[:, :, 1:515], HP[0:64, qlo:qhi, 0:514])
                            nc.sync.dma_start(lo2[:, :, 1:515], HP[64:128, qlo:qhi, 0:514])
                            nc.sync.dma_start(up1[:, :, 1:515], HP[0:64, qlo:qhi, 0:514])
                            nc.sync.dma_start(up2[:, :, 1:515], HP[64:128, qlo:qhi, 0:514])
                            # col-dup tile for conv2's shared-window matmul:
                            # rows (4m+1, 4m+2); lower = col-1, upper = col
                            nc.scalar.dma_start(Hc[0:64, qlo:qhi, 0, 1:257],
                                                HP[0:64, qlo:qhi, 258:514])
                            nc.scalar.dma_start(Hc[0:64, qlo:qhi, 1, 1:257],
                                                HP[64:128, qlo:qhi, 0:256])
                            nc.sync.dma_start(Hc[64:128, qlo:qhi, 0, 0:256],
                                              HP[0:64, qlo:qhi, 258:514])
                            nc.sync.dma_start(Hc[64:128, qlo:qhi, 1, 0:256],
                                              HP[64:128, qlo:qhi, 0:256])

                    if it >= 1:
                        # conv2 quad mm of strip sp
                        acc2 = ps2.tile([128, 512], dt.float32, tag="ps2")
                        l = 4 * mm
                        for dx in range(3):
                            nc.tensor.matmul(acc2[0:64, :], wa2[:, dx, :],
                                             Hsp[:, l:l + 2, dx:dx + 256],
                                             start=(dx == 0), stop=False,
                                             tile_position=(0, 0), skip_group_check=True)
                        for dx in range(3):
                            nc.tensor.matmul(acc2[64:128, :], wc2[:, dx, :],
                                             Hsp[:, l + 3:l + 5, dx:dx + 256],
                                             start=(dx == 0), stop=False,
                                             tile_position=(0, 64), skip_group_check=True)
                        nc.tensor.matmul(acc2[:, :], wbc2[:, :],
                                         Hc_t[sp][:, mm, :, 0:256],
                                         start=False, stop=False,
                                         tile_position=(0, 0), skip_group_check=True)
                        nc.tensor.matmul(acc2[:, :], wb2[:, :],
                                         Hsp[0:64, l + 2:l + 4, 2:258],
                                         start=False, stop=True,
                                         tile_position=(0, 0), skip_group_check=True)
                        u2 = upool.tile([128, 512], dt.bfloat16, tag="u2")
                        nc.scalar.activation(u2[:], acc2[:], IDENT,
                                             bias=sb2[:, 1:2], scale=sb2[:, 0:1])
                        t = tvpool.tile([128, 512], dt.bfloat16, tag="t")
                        nc.vector.tensor_tensor(t[:], u2[:], GQ[:, mm, :],
                                                mybir.AluOpType.mult)
                        v = tvpool.tile([128, 512], dt.bfloat16, tag="v")
                        nc.vector.tensor_tensor(
                            v[0:64, :].rearrange("p (r w) -> p r w", r=2),
                            t[0:64, :].rearrange("p (r w) -> p r w", r=2),
                            Xsp[0:64, l + 1:l + 3, 1:257], mybir.AluOpType.add)
                        nc.vector.tensor_tensor(
                            v[64:128, :].rearrange("p (r w) -> p r w", r=2),
                            t[64:128, :].rearrange("p (r w) -> p r w", r=2),
                            Xsp[64:128, l + 2:l + 4, 1:257], mybir.AluOpType.add)
                        nc.vector.tensor_scalar_max(OV[:, mm, :], v[:], 0.0)

                if it < NS and it >= 1:
                    # boundary: h[r0-1] from prev HP into this strip's lower slot 0
                    nc.sync.dma_start(Hs[0:64, 0:1, 1:257],
                                      HP_t[it - 1][64:128, NQ - 1:NQ, 258:514])

                if it >= 1:
                    # store strip sp: A rows (4q,4q+1) then B rows (4q+2,4q+3)
                    od = o_d[:, r0p:r0p + R, :].rearrange("c (q r) w -> c q (r w)", q=NQ)
                    if it == NS:
                        # last strip: per-quad stores (SP HWDGE; cheap issue)
                        # so the drain tail is short
                        for mm in range(NQ):
                            nc.sync.dma_start(od[:, mm:mm + 1, 0:512],
                                              OV[0:64, mm:mm + 1, :])
                            nc.sync.dma_start(od[:, mm:mm + 1, 512:1024],
                                              OV[64:128, mm:mm + 1, :])
                    else:
                        nc.gpsimd.dma_start(od[:, :, 0:512], OV[0:64, :, :])
                        nc.gpsimd.dma_start(od[:, :, 512:1024], OV[64:128, :, :])
    nc.finalize()
    return nc


def _host_prep(x, gate, w1, scale1, bias1, w2, scale2, bias2):
    # x_dup: [128, 260, 258] bf16 per image; slot g: lower = x[g-2], upper = x[g-1]
    xp = np.zeros((B, C, H + 5, WP), np.float32)
    xp[:, :, 2:2 + H, 1:257] = x
    x_dup = np.concatenate([xp[:, :, 0:H + 4, :], xp[:, :, 1:H + 5, :]], axis=1)
    x_dup = x_dup.astype(BF16)

    # gate quad layout: [128, 64, 512] fp8; 0/1 exact in fp8.
    # partitions 0:64 = gate rows (4m,4m+1); 64:128 = rows (4m+2,4m+3)
    g = gate[:, 0].reshape(B, H // 4, 4, W)
    ga = g[:, :, 0:2].reshape(B, H // 4, 512)
    gb = g[:, :, 2:4].reshape(B, H // 4, 512)
    gq_img = [np.ascontiguousarray(np.concatenate(
        [np.broadcast_to(ga[b][None], (64, H // 4, 512)),
         np.broadcast_to(gb[b][None], (64, H // 4, 512))], axis=0)).astype(BF16)
        for b in range(B)]

    # Xc: [128, 64, 2, 258]: rows (4m+1, 4m+2); lower half = x shifted +1 col
    # (tile col c = x col c-1), upper half = x col c
    xr = x.reshape(B, C, H // 4, 4, W)[:, :, :, 1:3, :]  # [B, C, 64, 2, 256]
    xc = np.zeros((B, 128, H // 4, 2, WP), np.float32)
    xc[:, 0:64, :, :, 1:257] = xr
    xc[:, 64:128, :, :, 0:256] = xr
    xc = xc.astype(BF16)

    def pack(w):
        wt = np.transpose(w, (1, 0, 2, 3))  # [ci, co, ky, kx]
        wa = np.empty((128, 3, 64), np.float32)
        wb = np.empty((64, 128), np.float32)
        wc = np.empty((128, 3, 64), np.float32)
        wbc = np.empty((128, 128), np.float32)
        for kx in range(3):
            wa[0:64, kx] = wt[:, :, 0, kx]
            wa[64:128, kx] = wt[:, :, 1, kx]
            wc[0:64, kx] = wt[:, :, 1, kx]
            wc[64:128, kx] = wt[:, :, 2, kx]
        wb[:, 0:64] = wt[:, :, 2, 2]
        wb[:, 64:128] = wt[:, :, 0, 2]
        wbc[0:64, 0:64] = wt[:, :, 2, 0]
        wbc[0:64, 64:128] = wt[:, :, 0, 0]
        wbc[64:128, 0:64] = wt[:, :, 2, 1]
        wbc[64:128, 64:128] = wt[:, :, 0, 1]
        return (wa.astype(BF16), wb.astype(BF16), wc.astype(BF16),
                wbc.astype(BF16))

    wa1, wb1, wc1, wbc1 = pack(w1)
    wa2, wb2, wc2, wbc2 = pack(w2)
    sb1 = np.stack([np.tile(scale1, 2), np.tile(bias1, 2)], axis=1).astype(np.float32)
    sb2 = np.stack([np.tile(scale2, 2), np.tile(bias2, 2)], axis=1).astype(np.float32)
    return (x_dup, xc, gq_img, wa1, wb1, wc1, wbc1, wa2, wb2, wc2, wbc2,
            sb1, sb2)


def kernel(x, gate, w1, scale1, bias1, w2, scale2, bias2):
    from concourse.bass_utils import run_bass_kernel_spmd

    x = np.asarray(x, np.float32)
    gate = np.asarray(gate, np.float32)
    (x_dup, xc, gq_img, wa1, wb1, wc1, wbc1, wa2, wb2, wc2, wbc2,
     sb1, sb2) = _host_prep(
        x, gate, np.asarray(w1, np.float32), np.asarray(scale1, np.float32),
        np.asarray(bias1, np.float32), np.asarray(w2, np.float32),
        np.asarray(scale2, np.float32), np.asarray(bias2, np.float32))

    if 'nc' not in _CACHE:
        _CACHE['nc'] = _build()
    nc = _CACHE['nc']

    in_maps = []
    for b in range(B):
        in_maps.append({
            "xd": np.ascontiguousarray(x_dup[b]),
            "xc": np.ascontiguousarray(xc[b]),
            "gq": gq_img[b],
            "wa1": wa1, "wb1": wb1, "wc1": wc1, "wbc1": wbc1,
            "wa2": wa2, "wb2": wb2, "wc2": wc2, "wbc2": wbc2,
            "sb1": sb1, "sb2": sb2,
        })
    res = run_bass_kernel_spmd(nc, in_maps, core_ids=list(range(B)))
    _CACHE['last_results'] = res
    out = np.stack([np.asarray(res.results[b]["o"]).astype(np.float32)
                    for b in range(B)], axis=0)
    return out


# revision 31
# speedup vs baseline: 1.1216x; 1.0097x over previous
"""Trainium2 Bass kernel for masked BasicBlock (conv3x3+BN+ReLU, gated, x2, residual).

Data-parallel over batch: 8 images -> 8 NeuronCores. Per core, NCHW [64,256,256]
in 8 row-strips of 32 rows.

Key algebraic fact: the gmax (maxpool3x3 of gate) multiply on h is redundant --
every pixel that survives the final `gate` multiply has all nine conv2-input
neighbors inside gmax==1, so unmasked h gives the identical output. So:
    h  = relu(bn1(conv1(x)))            (no mask)
    out= relu(bn2(conv2(h))*gate + x)

Each conv is computed per 4-row "quad" (pairs A=rows 4m..4m+1 on PSUM
partitions 0:64, B=rows 4m+2..4m+3 on partitions 64:128) with 9 matmuls
(the minimum for K<=128 packing):
  - input tiles hold (lower = row t, upper = row t+1) duplicated layouts so a
    K=128 matmul covers two adjacent row-taps at once;
  - per dx: (a) K=128 -> A (taps ky=0,1), (c) K=128 -> B (taps ky=1,2), and
    (b) K=64, M=128 covering A's ky=2 and B's ky=0 simultaneously.
x arrives from the host already in the shifted-dup bf16 layout; the gate
arrives broadcast to the quad layout (bf16, values 0/1, DVE 2x-mode mult);
conv1's h is rearranged into the dup layout with 2x4 strided SBUF DMAs per
strip (each run spans two adjacent Hs slots incl. interior zero pads) plus 2
boundary-row DMAs between neighbor strips (no halo recompute).
Output is stored bf16 and upcast to f32 on the host.
"""
import sys

sys.path.insert(0, '/opt/trn_rl_repo')

import numpy as np
import ml_dtypes

BF16 = ml_dtypes.bfloat16

B, C, H, W = 8, 64, 256, 256
WP = W + 2            # padded row width (zero col at 0 and 257)
R = 32                # rows per strip
NS = H // R           # strips
NQ = R // 4           # quads per strip
XSL = 33              # x_dup slots per strip
HSL = 35              # h dup slots per strip (33 used + 2 scratch for strided views)

_CACHE = {}


def _build():
    import concourse.bacc as bacc_mod
    import concourse.tile as tile
    import concourse.mybir as mybir

    dt = mybir.dt
    nc = bacc_mod.Bacc()

    xd = nc.dram_tensor("xd", [128, H + 4, WP], dt.bfloat16, kind="ExternalInput")
    xc_d = nc.dram_tensor("xc", [128, H // 4, 2, WP], dt.bfloat16, kind="ExternalInput")
    gq_d = nc.dram_tensor("gq", [128, H // 4, 512], dt.bfloat16, kind="ExternalInput")
    wa1_d = nc.dram_tensor("wa1", [128, 3, 64], dt.bfloat16, kind="ExternalInput")
    wb1_d = nc.dram_tensor("wb1", [64, 128], dt.bfloat16, kind="ExternalInput")
    wc1_d = nc.dram_tensor("wc1", [128, 3, 64], dt.bfloat16, kind="ExternalInput")
    wbc1_d = nc.dram_tensor("wbc1", [128, 128], dt.bfloat16, kind="ExternalInput")
    wa2_d = nc.dram_tensor("wa2", [128, 3, 64], dt.bfloat16, kind="ExternalInput")
    wb2_d = nc.dram_tensor("wb2", [64, 128], dt.bfloat16, kind="ExternalInput")
    wc2_d = nc.dram_tensor("wc2", [128, 3, 64], dt.bfloat16, kind="ExternalInput")
    wbc2_d = nc.dram_tensor("wbc2", [128, 128], dt.bfloat16, kind="ExternalInput")
    sb1_d = nc.dram_tensor("sb1", [128, 2], dt.float32, kind="ExternalInput")
    sb2_d = nc.dram_tensor("sb2", [128, 2], dt.float32, kind="ExternalInput")
    o_d = nc.dram_tensor("o", [C, H, W], dt.bfloat16, kind="ExternalOutput")

    RELU = mybir.ActivationFunctionType.Relu
    IDENT = mybir.ActivationFunctionType.Identity

    with tile.TileContext(nc) as tc:
        with (
            tc.tile_pool(name="const", bufs=1) as cpool,
            tc.tile_pool(name="xs", bufs=3) as xpool,
            tc.tile_pool(name="xc", bufs=2) as xcpool,
            tc.tile_pool(name="hs", bufs=2) as hpool,
            tc.tile_pool(name="hc", bufs=2) as hcpool,
            tc.tile_pool(name="hp", bufs=2) as hppool,
            tc.tile_pool(name="gq", bufs=2) as gqpool,
            tc.tile_pool(name="ov", bufs=2) as ovpool,
            tc.tile_pool(name="u2", bufs=2) as upool,
            tc.tile_pool(name="tv", bufs=3) as tvpool,
            tc.tile_pool(name="ps1", bufs=4, space="PSUM") as ps1,
            tc.tile_pool(name="ps2", bufs=4, space="PSUM") as ps2,
        ):
            wa1 = cpool.tile([128, 3, 64], dt.bfloat16)
            wb1 = cpool.tile([64, 128], dt.bfloat16)
            wc1 = cpool.tile([128, 3, 64], dt.bfloat16)
            wbc1 = cpool.tile([128, 128], dt.bfloat16)
            wa2 = cpool.tile([128, 3, 64], dt.bfloat16)
            wb2 = cpool.tile([64, 128], dt.bfloat16)
            wc2 = cpool.tile([128, 3, 64], dt.bfloat16)
            wbc2 = cpool.tile([128, 128], dt.bfloat16)
            sb1 = cpool.tile([128, 2], dt.float32)
            sb2 = cpool.tile([128, 2], dt.float32)
            # weights via SWDGE (Pool) so the first Xs load owns HWDGE at t=0
            for t, d in ((wa1, wa1_d), (wb1, wb1_d), (wc1, wc1_d), (wbc1, wbc1_d),
                         (wa2, wa2_d), (wb2, wb2_d), (wc2, wc2_d), (wbc2, wbc2_d),
                         (sb1, sb1_d), (sb2, sb2_d)):
                nc.gpsimd.dma_start(t[:], d[:])

            Xs_t = [None] * NS
            Xc_t = [None] * NS
            Hs_t = [None] * NS
            Hc_t = [None] * NS
            HP_t = [None] * NS

            for it in range(NS + 1):
                # ---------------- conv1 of strip s = it ----------------
                if it < NS:
                    s = it
                    r0 = s * R
                    Xs = xpool.tile([128, XSL, WP], dt.bfloat16, tag="Xs")
                    Xs_t[s] = Xs
                    if s == 0:
                        # chunked first load so quad-0 matmuls start early
                        for c0, c1 in ((0, 5), (5, 17), (17, XSL)):
                            nc.sync.dma_start(Xs[:, c0:c1, :],
                                              xd[:, r0 + 1 + c0:r0 + 1 + c1, :])
                    else:
                        nc.sync.dma_start(Xs[:], xd[:, r0 + 1:r0 + 1 + XSL, :])
                    # col-shift dup tile: rows (4m+1,4m+2) per quad, upper = +1 col
                    Xc = xcpool.tile([128, NQ, 2, WP], dt.bfloat16, tag="Xc")
                    Xc_t[s] = Xc
                    if s == 0:
                        for c0, c1 in ((0, 2), (2, NQ)):
                            nc.scalar.dma_start(Xc[:, c0:c1, :, :],
                                                xc_d[:, c0:c1, :, :])
                    else:
                        nc.scalar.dma_start(Xc[:], xc_d[:, s * NQ:(s + 1) * NQ, :, :])

                    Hs = hpool.tile([128, HSL, WP], dt.bfloat16, tag="Hs")
                    Hs_t[s] = Hs
                    nc.vector.memset(Hs[:, :, 0:1], 0)
                    nc.vector.memset(Hs[:, :, 257:258], 0)
                    Hc = hcpool.tile([128, NQ, 2, WP], dt.bfloat16, tag="Hc")
                    Hc_t[s] = Hc
                    nc.vector.memset(Hc[0:64, :, :, 0:1], 0)
                    nc.vector.memset(Hc[0:64, :, :, 257:258], 0)
                    nc.vector.memset(Hc[64:128, :, :, 256:258], 0)
                    if s == 0:
                        nc.vector.memset(Hs[0:64, 0:1, :], 0)       # h[-1] = 0
                    if s == NS - 1:
                        nc.vector.memset(Hs[64:128, 32:33, :], 0)   # h[256] = 0

                    # HP: per-quad staging with 2 zero gap cols between the
                    # row-pair halves so distribute DMA runs span 2 Hs slots
                    HP = hppool.tile([128, NQ, 516], dt.bfloat16, tag="HP")
                    HP_t[s] = HP
                    nc.vector.memset(HP[:, :, 256:258], 0)

                if it >= 1:
                    sp = it - 1
                    r0p = sp * R
                    GQ = gqpool.tile([128, NQ, 512], dt.bfloat16, tag="GQ")
                    nc.sync.dma_start(GQ[:], gq_d[:, sp * NQ:(sp + 1) * NQ, :])
                    OV = ovpool.tile([128, NQ, 512], dt.bfloat16, tag="OV")
                    Hsp = Hs_t[sp]
                    Xsp = Xs_t[sp]

                for mm in range(NQ):
                    if it < NS:
                        # conv1 quad mm: 9 matmuls into acc, then BN+relu -> HP
                        acc = ps1.tile([128, 512], dt.float32, tag="ps1")
                        l = 4 * mm
                        for dx in range(3):
                            nc.tensor.matmul(acc[0:64, :], wa1[:, dx, :],
                                             Xs[:, l:l + 2, dx:dx + 256],
                                             start=(dx == 0), stop=False,
                                             tile_position=(0, 0), skip_group_check=True)
                        for dx in range(3):
                            nc.tensor.matmul(acc[64:128, :], wc1[:, dx, :],
                                             Xs[:, l + 3:l + 5, dx:dx + 256],
                                             start=(dx == 0), stop=False,
                                             tile_position=(0, 64), skip_group_check=True)
                        # shared-window: col-dup 4-block matmul (kx0+kx1 for
                        # A-ky2 and B-ky0), then K=64 matmul for kx2
                        nc.tensor.matmul(acc[:, :], wbc1[:, :],
                                         Xc[:, mm, :, 0:256],
                                         start=False, stop=False,
                                         tile_position=(0, 0), skip_group_check=True)
                        nc.tensor.matmul(acc[:, :], wb1[:, :],
                                         Xs[0:64, l + 2:l + 4, 2:258],
                                         start=False, stop=True,
                                         tile_position=(0, 0), skip_group_check=True)
                        hpq = HP[:, mm, :].rearrange("p (r w) -> p r w", r=2)
                        nc.scalar.activation(hpq[:, :, 0:256], acc[:], RELU,
                                             bias=sb1[:, 1:2], scale=sb1[:, 0:1])
                        if mm == 0 and it >= 1:
                            # h[r0] -> prev strip's Hs upper slot 32
                            nc.sync.dma_start(Hs_t[it - 1][64:128, 32:33, 1:257],
                                              HP[0:64, 0:1, 0:256])
                        if mm == 3 or mm == 7:
                            # distribute this half-strip of HP into the Hs dup
                            # layout: 4 DMAs whose runs span 2 adjacent Hs slots
                            # (data + interior zero pads, 514 cols each)
                            qlo, qhi = (0, 4) if mm == 3 else (4, 8)
                            nq2 = qhi - qlo
                            lo1 = Hs[0:64, 1 + 4 * qlo:1 + 4 * qhi, :].rearrange(
                                "c (q r) w -> c q (r w)", q=nq2)
                            lo2 = Hs[0:64, 3 + 4 * qlo:3 + 4 * qhi, :].rearrange(
                                "c (q r) w -> c q (r w)", q=nq2)
                            up1 = Hs[64:128, 4 * qlo:4 * qhi, :].rearrange(
                                "c (q r) w -> c q (r w)", q=nq2)
                            up2 = Hs[64:128, 2 + 4 * qlo:2 + 4 * qhi, :].rearrange(
                                "c (q r) w -> c q (r w)", q=nq2)
                            nc.sync.dma_start(lo1[:, :, 1:515], HP[0:64, qlo:qhi, 0:514])
                            nc.sync.dma_start(lo2[:, :, 1:515], HP[64:128, qlo:qhi, 0:514])
                            nc.sync.dma_start(up1[:, :, 1:515], HP[0:64, qlo:qhi, 0:514])
                            nc.sync.dma_start(up2[:, :, 1:515], HP[64:128, qlo:qhi, 0:514])
                            # col-dup tile for conv2's shared-window matmul:
                            # rows (4m+1, 4m+2); lower = col-1, upper = col
                            nc.scalar.dma_start(Hc[0:64, qlo:qhi, 0, 1:257],
                                                HP[0:64, qlo:qhi, 258:514])
                            nc.scalar.dma_start(Hc[0:64, qlo:qhi, 1, 1:257],
                                                HP[64:128, qlo:qhi, 0:256])
                            nc.gpsimd.dma_start(Hc[64:128, qlo:qhi, 0, 0:256],
                                                HP[0:64, qlo:qhi, 258:514])
                            nc.gpsimd.dma_start(Hc[64:128, qlo:qhi, 1, 0:256],
                                                HP[64:128, qlo:qhi, 0:256])

                    if it >= 1:
                        # conv2 quad mm of strip sp
                        acc2 = ps2.tile([128, 512], dt.float32, tag="ps2")
                        l = 4 * mm
                        for dx in range(3):
                            nc.tensor.matmul(acc2[0:64, :], wa2[:, dx, :],
                                             Hsp[:, l:l + 2, dx:dx + 256],
                                             start=(dx == 0), stop=False,
                                             tile_position=(0, 0), skip_group_check=True)
                        for dx in range(3):
                            nc.tensor.matmul(acc2[64:128, :], wc2[:, dx, :],
                                             Hsp[:, l + 3:l + 5, dx:dx + 256],
                                             start=(dx == 0), stop=False,
                                             tile_position=(0, 64), skip_group_check=True)
                        nc.tensor.matmul(acc2[:, :], wbc2[:, :],
                                         Hc_t[sp][:, mm, :, 0:256],
                                         start=False, stop=False,
                                         tile_position=(0, 0), skip_group_check=True)
                        nc.tensor.matmul(acc2[:, :], wb2[:, :],
                                         Hsp[0:64, l + 2:l + 4, 2:258],
                                         start=False, stop=True,
                                         tile_position=(0, 0), skip_group_check=True)
                        u2 = upool.tile([128, 512], dt.bfloat16, tag="u2")
                        nc.scalar.activation(u2[:], acc2[:], IDENT,
                                             bias=sb2[:, 1:2], scale=sb2[:, 0:1])
                        t = tvpool.tile([128, 512], dt.bfloat16, tag="t")
                        nc.vector.tensor_tensor(t[:], u2[:], GQ[:, mm, :],
                                                mybir.AluOpType.mult)
                        v = tvpool.tile([128, 512], dt.bfloat16, tag="v")
                        nc.vector.tensor_tensor(
                            v[0:64, :].rearrange("p (r w) -> p r w", r=2),
                            t[0:64, :].rearrange("p (r w) -> p r w", r=2),
                            Xsp[0:64, l + 1:l + 3, 1:257], mybir.AluOpType.add)
                        nc.vector.tensor_tensor(
                            v[64:128, :].rearrange("p (r w) -> p r w", r=2),
                            t[64:128, :].rearrange("p (r w) -> p r w", r=2),
                            Xsp[64:128, l + 2:l + 4, 1:257], mybir.AluOpType.add)
                        nc.vector.tensor_scalar_max(OV[:, mm, :], v[:], 0.0)

                if it < NS and it >= 1:
                    # boundary: h[r0-1] from prev HP into this strip's lower slot 0
                    nc.sync.dma_start(Hs[0:64, 0:1, 1:257],
                                      HP_t[it - 1][64:128, NQ - 1:NQ, 258:514])

                if it >= 1:
                    # store strip sp: A rows (4q,4q+1) then B rows (4q+2,4q+3)
                    od = o_d[:, r0p:r0p + R, :].rearrange("c (q r) w -> c q (r w)", q=NQ)
                    if it == NS:
                        # last strip: per-quad stores (SP HWDGE; cheap issue)
                        # so the drain tail is short
                        for mm in range(NQ):
                            nc.sync.dma_start(od[:, mm:mm + 1, 0:512],
                                              OV[0:64, mm:mm + 1, :])
                            nc.sync.dma_start(od[:, mm:mm + 1, 512:1024],
                                              OV[64:128, mm:mm + 1, :])
                    else:
                        nc.gpsimd.dma_start(od[:, :, 0:512], OV[0:64, :, :])
                        nc.gpsimd.dma_start(od[:, :, 512:1024], OV[64:128, :, :])
    nc.finalize()
    return nc


def _host_prep(x, gate, w1, scale1, bias1, w2, scale2, bias2):
    # x_dup: [128, 260, 258] bf16 per image; slot g: lower = x[g-2], upper = x[g-1]
    xp = np.zeros((B, C, H + 5, WP), np.float32)
    xp[:, :, 2:2 + H, 1:257] = x
    x_dup = np.concatenate([xp[:, :, 0:H + 4, :], xp[:, :, 1:H + 5, :]], axis=1)
    x_dup = x_dup.astype(BF16)

    # gate quad layout: [128, 64, 512] fp8; 0/1 exact in fp8.
    # partitions 0:64 = gate rows (4m,4m+1); 64:128 = rows (4m+2,4m+3)
    g = gate[:, 0].reshape(B, H // 4, 4, W)
    ga = g[:, :, 0:2].reshape(B, H // 4, 512)
    gb = g[:, :, 2:4].reshape(B, H // 4, 512)
    gq_img = [np.ascontiguousarray(np.concatenate(
        [np.broadcast_to(ga[b][None], (64, H // 4, 512)),
         np.broadcast_to(gb[b][None], (64, H // 4, 512))], axis=0)).astype(BF16)
        for b in range(B)]

    # Xc: [128, 64, 2, 258]: rows (4m+1, 4m+2); lower half = x shifted +1 col
    # (tile col c = x col c-1), upper half = x col c
    xr = x.reshape(B, C, H // 4, 4, W)[:, :, :, 1:3, :]  # [B, C, 64, 2, 256]
    xc = np.zeros((B, 128, H // 4, 2, WP), np.float32)
    xc[:, 0:64, :, :, 1:257] = xr
    xc[:, 64:128, :, :, 0:256] = xr
    xc = xc.astype(BF16)

    def pack(w):
        wt = np.transpose(w, (1, 0, 2, 3))  # [ci, co, ky, kx]
        wa = np.empty((128, 3, 64), np.float32)
        wb = np.empty((64, 128), np.float32)
        wc = np.empty((128, 3, 64), np.float32)
        wbc = np.empty((128, 128), np.float32)
        for kx in range(3):
            wa[0:64, kx] = wt[:, :, 0, kx]
            wa[64:128, kx] = wt[:, :, 1, kx]
            wc[0:64, kx] = wt[:, :, 1, kx]
            wc[64:128, kx] = wt[:, :, 2, kx]
        wb[:, 0:64] = wt[:, :, 2, 2]
        wb[:, 64:128] = wt[:, :, 0, 2]
        wbc[0:64, 0:64] = wt[:, :, 2, 0]
        wbc[0:64, 64:128] = wt[:, :, 0, 0]
        wbc[64:128, 0:64] = wt[:, :, 2, 1]
        wbc[64:128, 64:128] = wt[:, :, 0, 1]
        return (wa.astype(BF16), wb.astype(BF16), wc.astype(BF16),
                wbc.astype(BF16))

    wa1, wb1, wc1, wbc1 = pack(w1)
    wa2, wb2, wc2, wbc2 = pack(w2)
    sb1 = np.stack([np.tile(scale1, 2), np.tile(bias1, 2)], axis=1).astype(np.float32)
    sb2 = np.stack([np.tile(scale2, 2), np.tile(bias2, 2)], axis=1).astype(np.float32)
    return (x_dup, xc, gq_img, wa1, wb1, wc1, wbc1, wa2, wb2, wc2, wbc2,
            sb1, sb2)


def kernel(x, gate, w1, scale1, bias1, w2, scale2, bias2):
    from concourse.bass_utils import run_bass_kernel_spmd

    x = np.asarray(x, np.float32)
    gate = np.asarray(gate, np.float32)
    (x_dup, xc, gq_img, wa1, wb1, wc1, wbc1, wa2, wb2, wc2, wbc2,
     sb1, sb2) = _host_prep(
        x, gate, np.asarray(w1, np.float32), np.asarray(scale1, np.float32),
        np.asarray(bias1, np.float32), np.asarray(w2, np.float32),
        np.asarray(scale2, np.float32), np.asarray(bias2, np.float32))

    if 'nc' not in _CACHE:
        _CACHE['nc'] = _build()
    nc = _CACHE['nc']

    in_maps = []
    for b in range(B):
        in_maps.append({
            "xd": np.ascontiguousarray(x_dup[b]),
            "xc": np.ascontiguousarray(xc[b]),
            "gq": gq_img[b],
            "wa1": wa1, "wb1": wb1, "wc1": wc1, "wbc1": wbc1,
            "wa2": wa2, "wb2": wb2, "wc2": wc2, "wbc2": wbc2,
            "sb1": sb1, "sb2": sb2,
        })
    res = run_bass_kernel_spmd(nc, in_maps, core_ids=list(range(B)))
    _CACHE['last_results'] = res
    out = np.stack([np.asarray(res.results[b]["o"]).astype(np.float32)
                    for b in range(B)], axis=0)
    return out
